# revision 21
# baseline (speedup 1.0000x reference)
"""Trainium2 Bass kernel for nn_CustomLoss_84043920048360 (V3).

Data-parallel over batch: 8 NeuronCores x 4 batches each, no collectives.

The loss reduces to per-batch segment-sums over positions s:
  Q[j, c]      = sum_{s: target[s]==j} x[s, c]
  counts[j, c] = sum_{s: target[s]==j} [argmax_c' x[s, c'] == c]
  sumexp[s]    = sum_c exp(x[s, c])

V3 device pipeline (per 2048-position iter, 16 chunks of 128):
  - x ships as bf16 (8.4 MB/core); onehot(target) ships as fp8e4 (4.2 MB/
    core, exact 0/1) and feeds the PE directly as the stationary operand.
  - DVE: rowmax via 3 rounds of pairwise tensor-tensor max (2x mode) then
    a small GPSIMD reduce; argmax-onehot compares split DVE/GPSIMD.
  - PE per chunk: transpose x chunk into PSUM (for the exp path) + two
    N=128 matmuls accumulating [Q | counts] with lhsT = onehot fp8.
  - ACT: one exp per iter (PSUM -> SBUF); PE N=1 ones-matmuls give sumexp.
  - Outputs (Q|counts|sumexp per batch) DMA out of PSUM directly.
The host does lse=log(sumexp), bincounts, mode=argmax(counts) (exact
tie-break), the cipher/nll formulas in float64, and the final combine.

Accuracy: counts/mode are exact except for bf16-argmax ties (~1.8% of
rows, washes out to ~2e-4 on the final scalar); Q/lse carry bf16 noise.

Position mapping: s = it*2048 + p*16 + g (p = SBUF partition, g =
chunk-in-iter) so each partition's DMA is one contiguous 4 KiB run.
"""

import numpy as np
import ml_dtypes

B, S, C = 32, 8192, 128
NCORES = 8
B_LOC = B // NCORES          # 4 batches per core
G = 16                       # chunks per iteration
CHUNK = 128                  # positions per chunk (matmul K)
ITERS = S // (G * CHUNK)     # 4 iterations per batch
QCW = 256 + ITERS * G        # 320: [Q | counts | sumexp cols]

_cache = {}


def _build(b_loc=B_LOC, iters=ITERS, wbufs=2, pbufs=2, n_pool=8,
           pool_red=False, evac_dma=False, tr_split=0):
    import concourse.bacc as bacc
    import concourse.tile as tile
    from concourse import mybir

    f32 = mybir.dt.float32
    bf16 = mybir.dt.bfloat16
    fp8 = mybir.dt.float8e4

    nc = bacc.Bacc(
        "TRN2", target_bir_lowering=False, debug=False, num_devices=NCORES
    )
    pred = nc.dram_tensor("predicted", [b_loc, iters, 128, G, CHUNK], bf16,
                          kind="ExternalInput")
    oht_in = nc.dram_tensor("oht_fp8", [b_loc, iters, 128, G, CHUNK], fp8,
                            kind="ExternalInput")
    ident = nc.dram_tensor("ident_bf16", [128, 128], bf16, kind="ExternalInput")
    qc_out = nc.dram_tensor("qc_out", [b_loc, 128, QCW], f32,
                            kind="ExternalOutput")

    AX = mybir.AxisListType.X
    EQ = mybir.AluOpType.is_equal
    MAX = mybir.AluOpType.max

    with tile.TileContext(nc) as tc:
        with (
            tc.tile_pool(name="consts", bufs=1) as consts,
            tc.tile_pool(name="inputs", bufs=3) as inputs,
            tc.tile_pool(name="work", bufs=wbufs) as work,
            tc.tile_pool(name="psum", bufs=pbufs, space="PSUM") as psum,
            tc.tile_pool(name="psumx", bufs=1, space="PSUM") as psumx,
        ):
            ident_sb = consts.tile([128, 128], bf16)
            ones_sb = consts.tile([128, 1], bf16)
            nc.vector.memset(ones_sb[:], 1.0)

            pending_evac = None
            for b in range(b_loc):
                xb = inputs.tile([128, iters, G, CHUNK], bf16, tag="xb")
                oh = inputs.tile([128, iters, G, CHUNK], fp8, tag="oh")
                pv = pred.ap()[b].rearrange("i p g c -> p i g c")
                ov = oht_in.ap()[b].rearrange("i p g c -> p i g c")
                # per-iter interleaved DMAs: x and onehot stream together so
                # neither starves the PE at batch boundaries
                for it in range(iters):
                    nc.sync.dma_start(xb[:, it], pv[:, it])
                    nc.sync.dma_start(oh[:, it], ov[:, it])
                    if b == 0 and it == 0:
                        # ident only needed by the first transpose; keep it
                        # off the startup critical path
                        nc.sync.dma_start(ident_sb[:], ident.ap())
                # Separate PSUM tiles padded to a full 2 KiB bank each: a
                # start=True matmul wipes open accumulations sharing its
                # bank, so q/c/se must never co-bank (verified on HW).
                q_ps = psum.tile([128, 512], f32, tag="q_ps")
                c_ps = psum.tile([128, 512], f32, tag="c_ps")
                se_ps = psum.tile([128, 512], f32, tag="se_ps")

                def emit_tail(it, oham8, ohamb, eT):
                    # counts- and sumexp-matmuls for iter `it`, emitted one
                    # iteration late so the PE never waits on DVE/Pool/ACT.
                    # Pool-built fp8 chunk pairs go through DoubleRow (2
                    # chunks per matmul at 0.5 cyc/row).
                    for g2 in range(0, n_pool, 2):
                        nc.tensor.matmul(
                            c_ps[:, 0:128],
                            oh[:, it, g2 : g2 + 2, :],
                            oham8[:, g2 : g2 + 2, :],
                            start=(it == 0 and g2 == 0),
                            stop=(it == iters - 1 and n_pool == G
                                  and g2 == G - 2),
                            perf_mode=mybir.MatmulPerfMode.DoubleRow,
                        )
                    for g in range(n_pool, G):
                        nc.tensor.matmul(
                            c_ps[:, 0:128], oh[:, it, g, :], ohamb[:, g, :],
                            start=(it == 0 and n_pool == 0 and g == 0),
                            stop=(it == iters - 1 and g == G - 1),
                        )
                    for g in range(G):
                        col = it * G + g
                        nc.tensor.matmul(
                            se_ps[:, col : col + 1], eT[:, g, :], ones_sb[:],
                            start=True, stop=True,
                        )

                prev = None
                for it in range(iters):
                    m64 = work.tile([128, G, 64], bf16, tag="m64")
                    m32 = work.tile([128, G, 32], bf16, tag="m32")
                    m16 = work.tile([128, G, 16], bf16, tag="m16")
                    rmax = work.tile([128, G], f32, tag="rmax")
                    if n_pool:
                        oham8 = work.tile([128, n_pool, CHUNK], fp8, tag="oham8")
                    else:
                        oham8 = None
                    ohamb = work.tile([128, G, CHUNK], bf16, tag="ohamb")
                    eT = work.tile([128, G, CHUNK], bf16, tag="eT")
                    # half-iter transpose buffers: [128, 8, 128] bf16 = one
                    # 2 KiB PSUM bank each
                    xT0 = psumx.tile([128, G // 2, CHUNK], bf16, tag="xT0")
                    xT1 = psumx.tile([128, G // 2, CHUNK], bf16, tag="xT1")

                    x_it = xb[:, it]
                    # rowmax: 3 pairwise-max rounds (2x) + small reduce
                    nc.vector.tensor_tensor(
                        out=m64[:], in0=x_it[:, :, 0:64], in1=x_it[:, :, 64:128],
                        op=MAX,
                    )
                    nc.vector.tensor_tensor(
                        out=m32[:], in0=m64[:, :, 0:32], in1=m64[:, :, 32:64],
                        op=MAX,
                    )
                    nc.vector.tensor_tensor(
                        out=m16[:], in0=m32[:, :, 0:16], in1=m32[:, :, 16:32],
                        op=MAX,
                    )
                    red_eng = nc.gpsimd if pool_red else nc.vector
                    red_eng.reduce_max(rmax[:], m16[:], axis=AX)

                    for g in range(G):
                        # onehot(argmax): compare, exact 0/1 out. Pool
                        # chunks emit fp8 (same Pool cost) for DoubleRow.
                        if g < n_pool:
                            nc.gpsimd.tensor_scalar(
                                out=oham8[:, g, :],
                                in0=x_it[:, g, :],
                                scalar1=rmax[:, g : g + 1],
                                scalar2=None,
                                op0=EQ,
                            )
                        else:
                            nc.vector.tensor_scalar(
                                out=ohamb[:, g, :],
                                in0=x_it[:, g, :],
                                scalar1=rmax[:, g : g + 1],
                                scalar2=None,
                                op0=EQ,
                            )
                        # transpose x chunk into PSUM for the exp path
                        xT = xT0 if g < G // 2 else xT1
                        nc.tensor.transpose(
                            xT[:, g % (G // 2), :], x_it[:, g, :], ident_sb[:]
                        )
                        # Q += onehot(t).T @ x
                        nc.tensor.matmul(
                            q_ps[:, 0:128], oh[:, it, g, :], x_it[:, g, :],
                            start=(it == 0 and g == 0),
                            stop=(it == iters - 1 and g == G - 1),
                        )
                        if g == G // 2 - 1:
                            # first half transposed: exp it while the second
                            # half streams through the PE
                            nc.scalar.activation(
                                eT[:, 0 : G // 2], xT0[:],
                                mybir.ActivationFunctionType.Exp,
                            )
                    if prev is not None:
                        emit_tail(*prev)
                    nc.scalar.activation(
                        eT[:, G // 2 : G], xT1[:],
                        mybir.ActivationFunctionType.Exp,
                    )
                    prev = (it, oham8, ohamb, eT)
                emit_tail(*prev)
                # evacuate via ACT; out-DMA issued from ACT's queue so the
                # SP queue stays free for input prefetch
                qsb = work.tile([128, QCW], f32, tag="qsb")
                nc.scalar.copy(qsb[:, 0:128], q_ps[:, 0:128])
                nc.scalar.copy(qsb[:, 128:256], c_ps[:, 0:128])
                nc.scalar.copy(qsb[:, 256:QCW], se_ps[:, 0 : QCW - 256])
                nc.scalar.dma_start(qc_out.ap()[b], qsb[:])

    nc.compile()
    return nc


def _get_nc():
    key = "v3"
    if key not in _cache:
        _cache[key] = _build()
    return _cache[key]


_BF16 = ml_dtypes.bfloat16
_FP8 = ml_dtypes.float8_e4m3
_IDENT = np.eye(128).astype(_BF16)
_EYE8 = np.eye(128).astype(_FP8)
last_results = None


def _run_device(predicted, target):
    """predicted [B,S,C] f32, target [B,S] int ->
    (q [B,128,128], counts [B,128,128], se [B,S]) float64"""
    from concourse.bass_utils import run_bass_kernel_spmd

    nc = _get_nc()
    xb = predicted.astype(_BF16)
    oh8 = _EYE8[target.astype(np.int64)]
    # s = it*2048 + p*16 + g  ->  [B, ITERS, 128, G, C]
    xb = xb.reshape(B, ITERS, 128, G, C)
    oh8 = oh8.reshape(B, ITERS, 128, G, C)
    in_maps = []
    for core in range(NCORES):
        b0 = core * B_LOC
        in_maps.append(
            {
                "predicted": np.ascontiguousarray(xb[b0 : b0 + B_LOC]),
                "oht_fp8": np.ascontiguousarray(oh8[b0 : b0 + B_LOC]),
                "ident_bf16": _IDENT,
            }
        )
    global last_results
    last_results = run_bass_kernel_spmd(nc, in_maps, core_ids=list(range(NCORES)))
    qc = np.concatenate([r["qc_out"] for r in last_results.results], axis=0)
    q = qc[:, :, 0:128]
    counts = qc[:, :, 128:256]
    # se[b, p, it*G+g] -> sumexp[b, s] with s = it*2048 + p*16 + g
    se = (
        qc[:, :, 256:QCW]
        .reshape(B, 128, ITERS, G)
        .transpose(0, 2, 1, 3)
        .reshape(B, S)
    )
    return q.astype(np.float64), counts.astype(np.float64), se.astype(np.float64)


def kernel(predicted, target):
    predicted = np.asarray(predicted)
    target = np.asarray(target)
    in_dtype = predicted.dtype
    q, counts, se = _run_device(predicted.astype(np.float32, copy=False), target)

    total_cipher = 0.0
    total_nz = 0
    total_gather = 0.0
    for b in range(B):
        Q = q[b]
        t_b = target[b].astype(np.int64)
        lse = np.log(se[b])
        n_eq = np.bincount(t_b, minlength=C).astype(np.float64)
        Lt = np.bincount(t_b, weights=lse, minlength=C)
        L = lse.sum()
        mode = np.argmax(counts[b], axis=1)
        P = Q.sum(axis=0)
        Qg = Q[np.arange(C), mode]
        Pg = P[mode]
        sum_all = L - Pg
        sum_eq = Lt - Qg
        sum_ne = sum_all - sum_eq
        ne_cnt = S - n_eq
        eq_mean = sum_eq / np.maximum(n_eq, 1.0)
        ne_mean = sum_ne / np.maximum(ne_cnt, 1.0)
        inv_ne = np.where(ne_cnt > 0, 1.0 / np.maximum(ne_mean, 1e-30), 0.0)
        cipher = np.where(n_eq > 0, 0.5 * eq_mean + 0.5 * inv_ne, 0.0)
        total_cipher += cipher.sum()
        total_nz += int((cipher != 0).sum())
        total_gather += Q[np.arange(C), np.arange(C)].sum()

    cipher_mean = total_cipher / max(total_nz, 1)
    nll = -total_gather / (B * S)
    out = 0.5 * cipher_mean + 0.5 * nll
    out_dtype = in_dtype if in_dtype in (np.float32, np.float64) else np.float32
    return np.asarray(out, dtype=out_dtype)


# revision 44
# speedup vs baseline: 1.0214x; 1.0214x over previous
"""Trainium2 Bass kernel for nn_CustomLoss_84043920048360 (V3).

Data-parallel over batch: 8 NeuronCores x 4 batches each, no collectives.

The loss reduces to per-batch segment-sums over positions s:
  Q[j, c]      = sum_{s: target[s]==j} x[s, c]
  counts[j, c] = sum_{s: target[s]==j} [argmax_c' x[s, c'] == c]
  sumexp[s]    = sum_c exp(x[s, c])

V3 device pipeline (per 2048-position iter, 16 chunks of 128):
  - x ships as bf16 (8.4 MB/core); onehot(target) ships as fp8e4 (4.2 MB/
    core, exact 0/1) and feeds the PE directly as the stationary operand.
  - DVE: rowmax via 3 rounds of pairwise tensor-tensor max (2x mode) then
    a small GPSIMD reduce; argmax-onehot compares split DVE/GPSIMD.
  - PE per chunk: transpose x chunk into PSUM (for the exp path) + two
    N=128 matmuls accumulating [Q | counts] with lhsT = onehot fp8.
  - ACT: one exp per iter (PSUM -> SBUF); PE N=1 ones-matmuls give sumexp.
  - Outputs (Q|counts|sumexp per batch) DMA out of PSUM directly.
The host does lse=log(sumexp), bincounts, mode=argmax(counts) (exact
tie-break), the cipher/nll formulas in float64, and the final combine.

Accuracy: counts/mode are exact except for bf16-argmax ties (~1.8% of
rows, washes out to ~2e-4 on the final scalar); Q/lse carry bf16 noise.

Position mapping: s = it*2048 + p*16 + g (p = SBUF partition, g =
chunk-in-iter) so each partition's DMA is one contiguous 4 KiB run.
"""

import numpy as np
import ml_dtypes

B, S, C = 32, 8192, 128
NCORES = 8
B_LOC = B // NCORES          # 4 batches per core
G = 16                       # chunks per iteration
CHUNK = 128                  # positions per chunk (matmul K)
ITERS = S // (G * CHUNK)     # 4 iterations per batch
QCW = 256 + ITERS * G        # 320: [Q | counts | sumexp cols]

_cache = {}


def _build(b_loc=B_LOC, iters=ITERS, wbufs=2, pbufs=2, n_pool=8,
           pool_red=False, evac_dma=False, tr_split=0):
    import concourse.bacc as bacc
    import concourse.tile as tile
    from concourse import mybir

    f32 = mybir.dt.float32
    bf16 = mybir.dt.bfloat16
    fp8 = mybir.dt.float8e4

    nc = bacc.Bacc(
        "TRN2", target_bir_lowering=False, debug=False, num_devices=NCORES
    )
    pred = nc.dram_tensor("predicted", [b_loc, iters, 128, G, CHUNK], bf16,
                          kind="ExternalInput")
    oht_in = nc.dram_tensor("oht_fp8", [b_loc, iters, 128, G, CHUNK], fp8,
                            kind="ExternalInput")
    ident = nc.dram_tensor("ident_bf16", [128, 128], bf16, kind="ExternalInput")
    qc_out = nc.dram_tensor("qc_out", [b_loc, 128, QCW], f32,
                            kind="ExternalOutput")

    AX = mybir.AxisListType.X
    EQ = mybir.AluOpType.is_equal
    MAX = mybir.AluOpType.max

    with tile.TileContext(nc) as tc:
        with (
            tc.tile_pool(name="consts", bufs=1) as consts,
            tc.tile_pool(name="inputs", bufs=3) as inputs,
            tc.tile_pool(name="work", bufs=wbufs) as work,
            tc.tile_pool(name="psum", bufs=1, space="PSUM") as psum,
        ):
            ident_sb = consts.tile([128, 128], bf16)
            ones_sb = consts.tile([128, 1], bf16)
            nc.vector.memset(ones_sb[:], 1.0)

            def emit_tail(ctx):
                # counts- and sumexp-matmuls for one iter, emitted one
                # iteration late (crossing batch boundaries) so the PE
                # never waits on DVE/Pool/ACT. Pool-built fp8 chunk pairs
                # go through DoubleRow (2 chunks per matmul, 0.5 cyc/row).
                oh_t, it, np_it, oham8, ohamb, eT, c_ps_t, se_ps_t = ctx
                c_start = it == 0
                c_stop = it == iters - 1
                for g2 in range(0, np_it, 2):
                    nc.tensor.matmul(
                        c_ps_t[:, 0:128],
                        oh_t[:, it, g2 : g2 + 2, :],
                        oham8[:, g2 : g2 + 2, :],
                        start=(c_start and g2 == 0),
                        stop=(c_stop and np_it == G and g2 == G - 2),
                        perf_mode=mybir.MatmulPerfMode.DoubleRow,
                    )
                for g in range(np_it, G):
                    nc.tensor.matmul(
                        c_ps_t[:, 0:128], oh_t[:, it, g, :], ohamb[:, g, :],
                        start=(c_start and np_it == 0 and g == 0),
                        stop=(c_stop and g == G - 1),
                    )
                for g in range(G):
                    col = it * G + g
                    nc.tensor.matmul(
                        se_ps_t[:, col : col + 1], eT[:, g, :], ones_sb[:],
                        start=True, stop=True,
                    )

            def prefetch(b):
                # allocate the batch's input tiles and issue their DMAs;
                # x and onehot interleave per-iter so neither starves the PE
                xb = inputs.tile([128, iters, G, CHUNK], bf16, tag="xb")
                oh = inputs.tile([128, iters, G, CHUNK], fp8, tag="oh")
                pv = pred.ap()[b].rearrange("i p g c -> p i g c")
                ov = oht_in.ap()[b].rearrange("i p g c -> p i g c")
                for it in range(iters):
                    nc.sync.dma_start(xb[:, it], pv[:, it])
                    if b == 0 and it == 0:
                        # tiny ident transfer slots between the first two
                        # input DMAs so transposes can start early
                        nc.sync.dma_start(ident_sb[:], ident.ap())
                    nc.sync.dma_start(oh[:, it], ov[:, it])
                return xb, oh

            def emit_maxes(xb_t, it):
                # rowmax of iter `it`: 3 pairwise-max rounds (2x) + a small
                # reduce. Runs one iteration AHEAD of its consumers so the
                # serial max chain never gates the compare stream.
                x_it = xb_t[:, it]
                m64 = work.tile([128, G, 64], bf16, tag="m64")
                m32 = work.tile([128, G, 32], bf16, tag="m32")
                m16 = work.tile([128, G, 16], bf16, tag="m16")
                rmax = work.tile([128, G], f32, tag="rmax")
                nc.vector.tensor_tensor(
                    out=m64[:], in0=x_it[:, :, 0:64], in1=x_it[:, :, 64:128],
                    op=MAX,
                )
                nc.vector.tensor_tensor(
                    out=m32[:], in0=m64[:, :, 0:32], in1=m64[:, :, 32:64],
                    op=MAX,
                )
                nc.vector.tensor_tensor(
                    out=m16[:], in0=m32[:, :, 0:16], in1=m32[:, :, 16:32],
                    op=MAX,
                )
                nc.vector.reduce_max(rmax[:], m16[:], axis=AX)
                return rmax

            pending_evac = None
            prev = None
            rmax_cur = None
            nxt = prefetch(0)
            for b in range(b_loc):
                xb, oh = nxt
                nxt = prefetch(b + 1) if b + 1 < b_loc else None
                # Separate PSUM tiles padded to a full 2 KiB bank each: a
                # start=True matmul wipes open accumulations sharing its
                # bank, so q/c/se must never co-bank (verified on HW).
                q_ps = psum.tile([128, 512], f32, tag="q_ps")
                c_ps = psum.tile([128, 512], f32, tag="c_ps")
                se_ps = psum.tile([128, 512], f32, tag="se_ps")

                for it in range(iters):
                    # ping-pong full-iter transpose buffers (2 banks each)
                    if (b * iters + it) % 2 == 0:
                        xTi = psum.tile([128, G, CHUNK], bf16, tag="xTa")
                    else:
                        xTi = psum.tile([128, G, CHUNK], bf16, tag="xTb")
                    # last iter overall: all compares on DVE, counts-matmuls
                    # inlined, so the drain tail is short
                    last = b == b_loc - 1 and it == iters - 1
                    np_it = 0 if last else n_pool
                    if n_pool:
                        oham8 = work.tile([128, n_pool, CHUNK], fp8, tag="oham8")
                    else:
                        oham8 = None
                    ohamb = work.tile([128, G, CHUNK], bf16, tag="ohamb")
                    eT = work.tile([128, G, CHUNK], bf16, tag="eT")

                    x_it = xb[:, it]
                    if rmax_cur is None:
                        rmax_cur = emit_maxes(xb, 0)
                    rmax = rmax_cur

                    for g in range(G):
                        # onehot(argmax): compare, exact 0/1 out. Pool
                        # chunks emit fp8 (same Pool cost) for DoubleRow.
                        if g < np_it:
                            nc.gpsimd.tensor_scalar(
                                out=oham8[:, g, :],
                                in0=x_it[:, g, :],
                                scalar1=rmax[:, g : g + 1],
                                scalar2=None,
                                op0=EQ,
                            )
                        else:
                            nc.vector.tensor_scalar(
                                out=ohamb[:, g, :],
                                in0=x_it[:, g, :],
                                scalar1=rmax[:, g : g + 1],
                                scalar2=None,
                                op0=EQ,
                            )
                        # transpose x chunk into PSUM for the exp path
                        nc.tensor.transpose(
                            xTi[:, g, :], x_it[:, g, :], ident_sb[:]
                        )
                        # Q += onehot(t).T @ x
                        nc.tensor.matmul(
                            q_ps[:, 0:128], oh[:, it, g, :], x_it[:, g, :],
                            start=(it == 0 and g == 0),
                            stop=(it == iters - 1 and g == G - 1),
                        )
                        if last and g >= 6:
                            # drain-tail shortening: inline counts-matmuls
                            # a few chunks behind their compares
                            gc = g - 6
                            nc.tensor.matmul(
                                c_ps[:, 0:128], oh[:, it, gc, :],
                                ohamb[:, gc, :],
                                start=False, stop=False,
                            )
                        if g == G // 2 - 1 and it == 1 and pending_evac is not None:
                            # previous batch's evacuation, emitted here so
                            # it never blocks the exp stream or prefetch
                            pending_evac()
                            pending_evac = None
                    # rowmax for the NEXT iter, pipelined one iter ahead but
                    # emitted after this iter's compares so it doesn't delay
                    # them on the in-order DVE queue
                    if it + 1 < iters:
                        rmax_cur = emit_maxes(xb, it + 1)
                    elif nxt is not None:
                        rmax_cur = emit_maxes(nxt[0], 0)
                    else:
                        rmax_cur = None
                    if prev is not None:
                        emit_tail(prev)
                        prev = None
                    # exp on the transposed tile (PSUM -> SBUF); runs on ACT
                    # during the next iter's chunk phase
                    nc.scalar.activation(
                        eT[:], xTi[:], mybir.ActivationFunctionType.Exp
                    )
                    if last:
                        for gc in range(G - 6, G):
                            nc.tensor.matmul(
                                c_ps[:, 0:128], oh[:, it, gc, :],
                                ohamb[:, gc, :],
                                start=False, stop=(gc == G - 1),
                            )
                        for g in range(G):
                            nc.tensor.matmul(
                                se_ps[:, it * G + g : it * G + g + 1],
                                eT[:, g, :], ones_sb[:], start=True, stop=True,
                            )
                    else:
                        prev = (oh, it, np_it, oham8, ohamb, eT, c_ps, se_ps)

                # evacuate via ACT; out-DMA issued from ACT's queue so the
                # SP queue stays free for input prefetch. The Q copy fires
                # now (its accumulation just closed); counts/sumexp copies
                # + the out-DMA are deferred into the next batch's second
                # iter (see pending_evac call site) so they never block.
                qsb = work.tile([128, QCW], f32, tag="qsb")
                nc.scalar.copy(qsb[:, 0:128], q_ps[:, 0:128])

                def make_evac(b=b, qsb=qsb, c_ps=c_ps, se_ps=se_ps):
                    def evac():
                        nc.scalar.copy(qsb[:, 128:256], c_ps[:, 0:128])
                        nc.scalar.copy(qsb[:, 256:QCW], se_ps[:, 0 : QCW - 256])
                        nc.scalar.dma_start(qc_out.ap()[b], qsb[:])
                    return evac

                if b < b_loc - 1:
                    pending_evac = make_evac()
                else:
                    make_evac()()

    nc.compile()
    return nc


def _get_nc():
    key = "v3"
    if key not in _cache:
        _cache[key] = _build()
    return _cache[key]


_BF16 = ml_dtypes.bfloat16
_FP8 = ml_dtypes.float8_e4m3
_IDENT = np.eye(128).astype(_BF16)
_EYE8 = np.eye(128).astype(_FP8)
last_results = None


def _run_device(predicted, target):
    """predicted [B,S,C] f32, target [B,S] int ->
    (q [B,128,128], counts [B,128,128], se [B,S]) float64"""
    from concourse.bass_utils import run_bass_kernel_spmd

    nc = _get_nc()
    xb = predicted.astype(_BF16)
    oh8 = _EYE8[target.astype(np.int64)]
    # s = it*2048 + p*16 + g  ->  [B, ITERS, 128, G, C]
    xb = xb.reshape(B, ITERS, 128, G, C)
    oh8 = oh8.reshape(B, ITERS, 128, G, C)
    in_maps = []
    for core in range(NCORES):
        b0 = core * B_LOC
        in_maps.append(
            {
                "predicted": np.ascontiguousarray(xb[b0 : b0 + B_LOC]),
                "oht_fp8": np.ascontiguousarray(oh8[b0 : b0 + B_LOC]),
                "ident_bf16": _IDENT,
            }
        )
    global last_results
    last_results = run_bass_kernel_spmd(nc, in_maps, core_ids=list(range(NCORES)))
    qc = np.concatenate([r["qc_out"] for r in last_results.results], axis=0)
    q = qc[:, :, 0:128]
    counts = qc[:, :, 128:256]
    # se[b, p, it*G+g] -> sumexp[b, s] with s = it*2048 + p*16 + g
    se = (
        qc[:, :, 256:QCW]
        .reshape(B, 128, ITERS, G)
        .transpose(0, 2, 1, 3)
        .reshape(B, S)
    )
    return q.astype(np.float64), counts.astype(np.float64), se.astype(np.float64)


def kernel(predicted, target):
    predicted = np.asarray(predicted)
    target = np.asarray(target)
    in_dtype = predicted.dtype
    q, counts, se = _run_device(predicted.astype(np.float32, copy=False), target)

    total_cipher = 0.0
    total_nz = 0
    total_gather = 0.0
    for b in range(B):
        Q = q[b]
        t_b = target[b].astype(np.int64)
        lse = np.log(se[b])
        n_eq = np.bincount(t_b, minlength=C).astype(np.float64)
        Lt = np.bincount(t_b, weights=lse, minlength=C)
        L = lse.sum()
        mode = np.argmax(counts[b], axis=1)
        P = Q.sum(axis=0)
        Qg = Q[np.arange(C), mode]
        Pg = P[mode]
        sum_all = L - Pg
        sum_eq = Lt - Qg
        sum_ne = sum_all - sum_eq
        ne_cnt = S - n_eq
        eq_mean = sum_eq / np.maximum(n_eq, 1.0)
        ne_mean = sum_ne / np.maximum(ne_cnt, 1.0)
        inv_ne = np.where(ne_cnt > 0, 1.0 / np.maximum(ne_mean, 1e-30), 0.0)
        cipher = np.where(n_eq > 0, 0.5 * eq_mean + 0.5 * inv_ne, 0.0)
        total_cipher += cipher.sum()
        total_nz += int((cipher != 0).sum())
        total_gather += Q[np.arange(C), np.arange(C)].sum()

    cipher_mean = total_cipher / max(total_nz, 1)
    nll = -total_gather / (B * S)
    out = 0.5 * cipher_mean + 0.5 * nll
    out_dtype = in_dtype if in_dtype in (np.float32, np.float64) else np.float32
    return np.asarray(out, dtype=out_dtype)


# revision 52
# speedup vs baseline: 1.0401x; 1.0184x over previous
"""Trainium2 Bass kernel for nn_CustomLoss_84043920048360 (V3).

Data-parallel over batch: 8 NeuronCores x 4 batches each, no collectives.

The loss reduces to per-batch segment-sums over positions s:
  Q[j, c]      = sum_{s: target[s]==j} x[s, c]
  counts[j, c] = sum_{s: target[s]==j} [argmax_c' x[s, c'] == c]
  sumexp[s]    = sum_c exp(x[s, c])

V3 device pipeline (per 2048-position iter, 16 chunks of 128):
  - x ships as bf16 (8.4 MB/core); onehot(target) ships as fp8e4 (4.2 MB/
    core, exact 0/1) and feeds the PE directly as the stationary operand.
  - DVE: rowmax via 3 rounds of pairwise tensor-tensor max (2x mode) then
    a small GPSIMD reduce; argmax-onehot compares split DVE/GPSIMD.
  - PE per chunk: transpose x chunk into PSUM (for the exp path) + two
    N=128 matmuls accumulating [Q | counts] with lhsT = onehot fp8.
  - ACT: one exp per iter (PSUM -> SBUF); PE N=1 ones-matmuls give sumexp.
  - Outputs (Q|counts|sumexp per batch) DMA out of PSUM directly.
The host does lse=log(sumexp), bincounts, mode=argmax(counts) (exact
tie-break), the cipher/nll formulas in float64, and the final combine.

Accuracy: counts/mode are exact except for bf16-argmax ties (~1.8% of
rows, washes out to ~2e-4 on the final scalar); Q/lse carry bf16 noise.

Position mapping: s = it*2048 + p*16 + g (p = SBUF partition, g =
chunk-in-iter) so each partition's DMA is one contiguous 4 KiB run.
"""

import numpy as np
import ml_dtypes

B, S, C = 32, 8192, 128
NCORES = 8
B_LOC = B // NCORES          # 4 batches per core
G = 16                       # chunks per iteration
CHUNK = 128                  # positions per chunk (matmul K)
ITERS = S // (G * CHUNK)     # 4 iterations per batch
QCW = 256 + ITERS * G        # 320: [Q | counts | sumexp cols]

_cache = {}


def _build(b_loc=B_LOC, iters=ITERS, wbufs=4, pbufs=2, n_pool=6,
           pool_red=False, evac_dma=False, tr_split=0):
    import concourse.bacc as bacc
    import concourse.tile as tile
    from concourse import mybir

    f32 = mybir.dt.float32
    bf16 = mybir.dt.bfloat16
    fp8 = mybir.dt.float8e4

    nc = bacc.Bacc(
        "TRN2", target_bir_lowering=False, debug=False, num_devices=NCORES
    )
    pred = nc.dram_tensor("predicted", [b_loc, iters, 128, G, CHUNK], bf16,
                          kind="ExternalInput")
    oht_in = nc.dram_tensor("oht_fp8", [b_loc, iters, 128, G, CHUNK], fp8,
                            kind="ExternalInput")
    ident = nc.dram_tensor("ident_bf16", [128, 128], bf16, kind="ExternalInput")
    qc_out = nc.dram_tensor("qc_out", [b_loc, 128, QCW], f32,
                            kind="ExternalOutput")

    AX = mybir.AxisListType.X
    EQ = mybir.AluOpType.is_equal
    MAX = mybir.AluOpType.max

    with tile.TileContext(nc) as tc:
        with (
            tc.tile_pool(name="consts", bufs=1) as consts,
            tc.tile_pool(name="inputs", bufs=3) as inputs,
            tc.tile_pool(name="work", bufs=wbufs) as work,
            tc.tile_pool(name="psum", bufs=1, space="PSUM") as psum,
        ):
            ident_sb = consts.tile([128, 128], bf16)
            ones_sb = consts.tile([128, 1], bf16)
            nc.vector.memset(ones_sb[:], 1.0)

            def emit_tail(ctx):
                # counts- and sumexp-matmuls for one iter, emitted one
                # iteration late (crossing batch boundaries) so the PE
                # never waits on DVE/Pool/ACT. Pool-built fp8 chunk pairs
                # go through DoubleRow (2 chunks per matmul, 0.5 cyc/row).
                oh_t, it, np_it, oham8, ohamb, eT, c_ps_t, se_ps_t = ctx
                c_start = it == 0
                c_stop = it == iters - 1
                for g2 in range(0, np_it, 2):
                    nc.tensor.matmul(
                        c_ps_t[:, 0:128],
                        oh_t[:, it, g2 : g2 + 2, :],
                        oham8[:, g2 : g2 + 2, :],
                        start=(c_start and g2 == 0),
                        stop=(c_stop and np_it == G and g2 == G - 2),
                        perf_mode=mybir.MatmulPerfMode.DoubleRow,
                    )
                for g in range(np_it, G):
                    nc.tensor.matmul(
                        c_ps_t[:, 0:128], oh_t[:, it, g, :], ohamb[:, g, :],
                        start=(c_start and np_it == 0 and g == 0),
                        stop=(c_stop and g == G - 1),
                    )
                for g in range(G):
                    col = it * G + g
                    nc.tensor.matmul(
                        se_ps_t[:, col : col + 1], eT[:, g, :], ones_sb[:],
                        start=True, stop=True,
                    )

            def prefetch(b):
                # allocate the batch's input tiles and issue their DMAs;
                # x and onehot interleave per-iter so neither starves the PE
                xb = inputs.tile([128, iters, G, CHUNK], bf16, tag="xb")
                oh = inputs.tile([128, iters, G, CHUNK], fp8, tag="oh")
                pv = pred.ap()[b].rearrange("i p g c -> p i g c")
                ov = oht_in.ap()[b].rearrange("i p g c -> p i g c")
                for it in range(iters):
                    nc.sync.dma_start(xb[:, it], pv[:, it])
                    if b == 0 and it == 0:
                        # tiny ident transfer slots between the first two
                        # input DMAs so transposes can start early
                        nc.sync.dma_start(ident_sb[:], ident.ap())
                    nc.sync.dma_start(oh[:, it], ov[:, it])
                return xb, oh

            def emit_maxes(xb_t, it):
                # rowmax of iter `it`: 3 pairwise-max rounds (2x) + a small
                # reduce. Runs one iteration AHEAD of its consumers so the
                # serial max chain never gates the compare stream.
                x_it = xb_t[:, it]
                m64 = work.tile([128, G, 64], bf16, tag="m64")
                m32 = work.tile([128, G, 32], bf16, tag="m32")
                m16 = work.tile([128, G, 16], bf16, tag="m16")
                rmax = work.tile([128, G], f32, tag="rmax")
                nc.vector.tensor_tensor(
                    out=m64[:], in0=x_it[:, :, 0:64], in1=x_it[:, :, 64:128],
                    op=MAX,
                )
                nc.vector.tensor_tensor(
                    out=m32[:], in0=m64[:, :, 0:32], in1=m64[:, :, 32:64],
                    op=MAX,
                )
                nc.vector.tensor_tensor(
                    out=m16[:], in0=m32[:, :, 0:16], in1=m32[:, :, 16:32],
                    op=MAX,
                )
                nc.vector.reduce_max(rmax[:], m16[:], axis=AX)
                return rmax

            pending_evac = None
            prev = None
            rmax_cur = None
            nxt = prefetch(0)
            for b in range(b_loc):
                xb, oh = nxt
                nxt = prefetch(b + 1) if b + 1 < b_loc else None
                # Separate PSUM tiles padded to a full 2 KiB bank each: a
                # start=True matmul wipes open accumulations sharing its
                # bank, so q/c/se must never co-bank (verified on HW).
                q_ps = psum.tile([128, 512], f32, tag="q_ps")
                c_ps = psum.tile([128, 512], f32, tag="c_ps")
                se_ps = psum.tile([128, 512], f32, tag="se_ps")

                for it in range(iters):
                    # ping-pong full-iter transpose buffers (2 banks each)
                    if (b * iters + it) % 2 == 0:
                        xTi = psum.tile([128, G, CHUNK], bf16, tag="xTa")
                    else:
                        xTi = psum.tile([128, G, CHUNK], bf16, tag="xTb")
                    # last iter overall: all compares on DVE, counts-matmuls
                    # inlined, so the drain tail is short
                    last = b == b_loc - 1 and it == iters - 1
                    np_it = 0 if last else n_pool
                    if n_pool:
                        oham8 = work.tile([128, n_pool, CHUNK], fp8, tag="oham8")
                    else:
                        oham8 = None
                    ohamb = work.tile([128, G, CHUNK], bf16, tag="ohamb")
                    eT = work.tile([128, G, CHUNK], bf16, tag="eT")

                    x_it = xb[:, it]
                    if rmax_cur is None:
                        rmax_cur = emit_maxes(xb, 0)
                    rmax = rmax_cur

                    if last:
                        # hoist the final iter's transposes so its exp (the
                        # drain-critical op) starts as early as possible
                        for g in range(G):
                            nc.tensor.transpose(
                                xTi[:, g, :], x_it[:, g, :], ident_sb[:]
                            )
                        nc.scalar.activation(
                            eT[:], xTi[:], mybir.ActivationFunctionType.Exp
                        )
                    for g in range(G):
                        # onehot(argmax): compare, exact 0/1 out. Pool
                        # chunks emit fp8 (same Pool cost) for DoubleRow.
                        if g < np_it:
                            nc.gpsimd.tensor_scalar(
                                out=oham8[:, g, :],
                                in0=x_it[:, g, :],
                                scalar1=rmax[:, g : g + 1],
                                scalar2=None,
                                op0=EQ,
                            )
                        else:
                            nc.vector.tensor_scalar(
                                out=ohamb[:, g, :],
                                in0=x_it[:, g, :],
                                scalar1=rmax[:, g : g + 1],
                                scalar2=None,
                                op0=EQ,
                            )
                        # transpose x chunk into PSUM for the exp path
                        if not last:
                            nc.tensor.transpose(
                                xTi[:, g, :], x_it[:, g, :], ident_sb[:]
                            )
                        # Q += onehot(t).T @ x
                        nc.tensor.matmul(
                            q_ps[:, 0:128], oh[:, it, g, :], x_it[:, g, :],
                            start=(it == 0 and g == 0),
                            stop=(it == iters - 1 and g == G - 1),
                        )
                        if last and g >= 6:
                            # drain-tail shortening: inline counts-matmuls
                            # a few chunks behind their compares
                            gc = g - 6
                            nc.tensor.matmul(
                                c_ps[:, 0:128], oh[:, it, gc, :],
                                ohamb[:, gc, :],
                                start=False, stop=False,
                            )
                        if g == G // 2 - 1 and it == 1 and pending_evac is not None:
                            # previous batch's evacuation, emitted here so
                            # it never blocks the exp stream or prefetch
                            pending_evac()
                            pending_evac = None
                    # rowmax for the NEXT iter, pipelined one iter ahead but
                    # emitted after this iter's compares so it doesn't delay
                    # them on the in-order DVE queue
                    if it + 1 < iters:
                        rmax_cur = emit_maxes(xb, it + 1)
                    elif nxt is not None:
                        rmax_cur = emit_maxes(nxt[0], 0)
                    else:
                        rmax_cur = None
                    if prev is not None:
                        emit_tail(prev)
                        prev = None
                    if not last:
                        # exp on the transposed tile (PSUM -> SBUF); runs on
                        # ACT during the next iter's chunk phase
                        nc.scalar.activation(
                            eT[:], xTi[:], mybir.ActivationFunctionType.Exp
                        )
                    if last:
                        for gc in range(G - 6, G):
                            nc.tensor.matmul(
                                c_ps[:, 0:128], oh[:, it, gc, :],
                                ohamb[:, gc, :],
                                start=False, stop=(gc == G - 1),
                            )
                        for g in range(G):
                            nc.tensor.matmul(
                                se_ps[:, it * G + g : it * G + g + 1],
                                eT[:, g, :], ones_sb[:], start=True, stop=True,
                            )
                    else:
                        prev = (oh, it, np_it, oham8, ohamb, eT, c_ps, se_ps)

                # evacuate via ACT; out-DMA issued from ACT's queue so the
                # SP queue stays free for input prefetch. The Q copy fires
                # now (its accumulation just closed); counts/sumexp copies
                # + the out-DMA are deferred into the next batch's second
                # iter (see pending_evac call site) so they never block.
                qsb = work.tile([128, QCW], f32, tag="qsb")
                nc.scalar.copy(qsb[:, 0:128], q_ps[:, 0:128])

                def make_evac(b=b, qsb=qsb, c_ps=c_ps, se_ps=se_ps):
                    def evac():
                        nc.scalar.copy(qsb[:, 128:256], c_ps[:, 0:128])
                        nc.scalar.copy(qsb[:, 256:QCW], se_ps[:, 0 : QCW - 256])
                        nc.scalar.dma_start(qc_out.ap()[b], qsb[:])
                    return evac

                if b < b_loc - 1:
                    pending_evac = make_evac()
                else:
                    make_evac()()

    nc.compile()
    return nc


def _get_nc():
    key = "v3"
    if key not in _cache:
        _cache[key] = _build()
    return _cache[key]


_BF16 = ml_dtypes.bfloat16
_FP8 = ml_dtypes.float8_e4m3
_IDENT = np.eye(128).astype(_BF16)
_EYE8 = np.eye(128).astype(_FP8)
last_results = None


def _run_device(predicted, target):
    """predicted [B,S,C] f32, target [B,S] int ->
    (q [B,128,128], counts [B,128,128], se [B,S]) float64"""
    from concourse.bass_utils import run_bass_kernel_spmd

    nc = _get_nc()
    xb = predicted.astype(_BF16)
    oh8 = _EYE8[target.astype(np.int64)]
    # s = it*2048 + p*16 + g  ->  [B, ITERS, 128, G, C]
    xb = xb.reshape(B, ITERS, 128, G, C)
    oh8 = oh8.reshape(B, ITERS, 128, G, C)
    in_maps = []
    for core in range(NCORES):
        b0 = core * B_LOC
        in_maps.append(
            {
                "predicted": np.ascontiguousarray(xb[b0 : b0 + B_LOC]),
                "oht_fp8": np.ascontiguousarray(oh8[b0 : b0 + B_LOC]),
                "ident_bf16": _IDENT,
            }
        )
    global last_results
    last_results = run_bass_kernel_spmd(nc, in_maps, core_ids=list(range(NCORES)))
    qc = np.concatenate([r["qc_out"] for r in last_results.results], axis=0)
    q = qc[:, :, 0:128]
    counts = qc[:, :, 128:256]
    # se[b, p, it*G+g] -> sumexp[b, s] with s = it*2048 + p*16 + g
    se = (
        qc[:, :, 256:QCW]
        .reshape(B, 128, ITERS, G)
        .transpose(0, 2, 1, 3)
        .reshape(B, S)
    )
    return q.astype(np.float64), counts.astype(np.float64), se.astype(np.float64)


def kernel(predicted, target):
    predicted = np.asarray(predicted)
    target = np.asarray(target)
    in_dtype = predicted.dtype
    q, counts, se = _run_device(predicted.astype(np.float32, copy=False), target)

    total_cipher = 0.0
    total_nz = 0
    total_gather = 0.0
    for b in range(B):
        Q = q[b]
        t_b = target[b].astype(np.int64)
        lse = np.log(se[b])
        n_eq = np.bincount(t_b, minlength=C).astype(np.float64)
        Lt = np.bincount(t_b, weights=lse, minlength=C)
        L = lse.sum()
        mode = np.argmax(counts[b], axis=1)
        P = Q.sum(axis=0)
        Qg = Q[np.arange(C), mode]
        Pg = P[mode]
        sum_all = L - Pg
        sum_eq = Lt - Qg
        sum_ne = sum_all - sum_eq
        ne_cnt = S - n_eq
        eq_mean = sum_eq / np.maximum(n_eq, 1.0)
        ne_mean = sum_ne / np.maximum(ne_cnt, 1.0)
        inv_ne = np.where(ne_cnt > 0, 1.0 / np.maximum(ne_mean, 1e-30), 0.0)
        cipher = np.where(n_eq > 0, 0.5 * eq_mean + 0.5 * inv_ne, 0.0)
        total_cipher += cipher.sum()
        total_nz += int((cipher != 0).sum())
        total_gather += Q[np.arange(C), np.arange(C)].sum()

    cipher_mean = total_cipher / max(total_nz, 1)
    nll = -total_gather / (B * S)
    out = 0.5 * cipher_mean + 0.5 * nll
    out_dtype = in_dtype if in_dtype in (np.float32, np.float64) else np.float32
    return np.asarray(out, dtype=out_dtype)
